# revision 40
# baseline (speedup 1.0000x reference)
"""Trainium2 Bass kernel for CTAttention (ragged-batch multi-head attention).

Host: pads/scatters ragged rows into [B, NMAX, C], shards batch elements
across 8 NeuronCores (batch b -> core b), compresses the additive key mask
[B, N, N] (rows are constant along the query axis) to a per-key vector.

Device (per core, one batch element):
  QKV^T = Waug^T @ x_aug        (bias folded via a ones-row, K pre-scaled)
  S^T[k, q] = K^T.T-slice @ Q^T (per head, per 128-key tile)
  P~^T = exp(S^T + mask[k])     (ScalarE, mask as per-partition bias)
  O'^T = V_aug.T @ P~^T         (V carries a ones column -> row 32 = rowsum)
  O^T = O'^T * recip(rowsum)    (selector-matmul partition broadcast)
  out = O_aug @ Wp_aug          (bias folded via the ones-row)
Matmuls run in float32r (TF32-like, full-rate fp32).
"""

import sys

sys.path.insert(0, "/opt/trn_rl_repo")

import numpy as np

B = 8
NMAX = 1024
C = 256
H = 8
HD = C // H
SCALE = HD ** -0.5

_CACHE = {}


def _build_program():
    import concourse.bass as bass
    from concourse import bacc
    import concourse.mybir as mybir
    import concourse.tile as tile

    F32 = mybir.dt.float32
    F32R = mybir.dt.float32r
    Exp = mybir.ActivationFunctionType.Exp

    nc = bacc.Bacc()

    xT0_d = nc.dram_tensor("xT0", [128, NMAX], F32, kind="ExternalInput")
    xT1_d = nc.dram_tensor("xT1", [128, NMAX], F32, kind="ExternalInput")
    ones_d = nc.dram_tensor("ones_row", [1, NMAX], F32, kind="ExternalInput")
    wqk0_d = nc.dram_tensor("wqk0", [128, 512], F32, kind="ExternalInput")
    wqk1_d = nc.dram_tensor("wqk1", [128, 512], F32, kind="ExternalInput")
    wqkb_d = nc.dram_tensor("wqkb", [1, 512], F32, kind="ExternalInput")
    wv0_d = nc.dram_tensor("wv0", [128, 264], F32, kind="ExternalInput")
    wv1_d = nc.dram_tensor("wv1", [128, 264], F32, kind="ExternalInput")
    wvb_d = nc.dram_tensor("wvb", [1, 264], F32, kind="ExternalInput")
    wpa_d = nc.dram_tensor("wpa", [C + 1, C], F32, kind="ExternalInput")
    maskc_d = nc.dram_tensor("maskc", [128, 8], F32, kind="ExternalInput")
    bones_d = nc.dram_tensor("bones", [33, 32], F32, kind="ExternalInput")
    out_d = nc.dram_tensor("out", [NMAX, C], F32, kind="ExternalOutput")

    with tile.TileContext(nc) as tc:
        with (
            nc.allow_low_precision("float32r pipeline; verified vs reference"),
            tc.tile_pool(name="const", bufs=1) as cpool,
            tc.tile_pool(name="qk", bufs=1) as qkpool,
            tc.tile_pool(name="vp", bufs=1) as vpool,
            tc.tile_pool(name="pt", bufs=2) as ppool,
            tc.tile_pool(name="orw", bufs=2) as orawpool,
            tc.tile_pool(name="ot", bufs=1) as opool,
            tc.tile_pool(name="stg", bufs=1) as stgpool,
            tc.tile_pool(name="io", bufs=4) as iopool,
            tc.tile_pool(name="ps_s", bufs=2, space="PSUM") as ps_s,
            tc.tile_pool(name="ps_o", bufs=2, space="PSUM") as ps_o,
            tc.tile_pool(name="ps_m", bufs=2, space="PSUM") as ps_m,
        ):
            # ---- constants / inputs (DMA order: QK-group-0 critical path first) ----
            xT0 = cpool.tile([128, NMAX], F32R)
            xT1 = cpool.tile([128, NMAX], F32R)
            ones_row = cpool.tile([1, NMAX], F32R)
            wqk0 = cpool.tile([128, 512], F32R)
            wqk1 = cpool.tile([128, 512], F32R)
            wqkb = cpool.tile([1, 512], F32R)
            wv0 = cpool.tile([128, 264], F32R)
            wv1 = cpool.tile([128, 264], F32R)
            wvb = cpool.tile([1, 264], F32R)
            wpc = [cpool.tile([32, C], F32R, name=f"wpc{i}") for i in range(8)]
            wpb = cpool.tile([1, C], F32R)
            maskc = cpool.tile([128, 8], F32)
            bones = cpool.tile([33, 32], F32R)
            nc.scalar.dma_start(xT0[:], xT0_d[:].bitcast(F32R))
            for eng, t, d in (
                (nc.sync, wqk0, wqk0_d),
                (nc.sync, wqk1, wqk1_d), (nc.scalar, xT1, xT1_d),
                (nc.sync, wqkb, wqkb_d), (nc.scalar, ones_row, ones_d),
                (nc.gpsimd, wv0, wv0_d), (nc.gpsimd, wv1, wv1_d),
                (nc.gpsimd, wvb, wvb_d), (nc.sync, bones, bones_d),
            ):
                eng.dma_start(t[:], d[:].bitcast(F32R))
            nc.scalar.dma_start(maskc[:], maskc_d[:])
            for i in range(8):
                nc.gpsimd.dma_start(
                    wpc[i][:], wpa_d[32 * i : 32 * i + 32, :].bitcast(F32R))
            nc.gpsimd.dma_start(wpb[:], wpa_d[C : C + 1, :].bitcast(F32R))

            qkT = [qkpool.tile([128, NMAX], F32R, name=f"qkT{i}") for i in range(4)]
            v_sb = [vpool.tile([128, 264], F32R, name=f"v{i}") for i in range(8)]
            oTh = [opool.tile([32, NMAX], F32R, name=f"oTh{i}") for i in range(H)]

            def emit_qk(co_t, j, skip128=False):
                pq = ps_m.tile([128, 512], F32, tag="m")
                js = slice(j * 512, (j + 1) * 512)
                cs = slice(co_t * 128, (co_t + 1) * 128)
                nc.tensor.matmul(pq[:], wqk0[:, cs], xT0[:, js], start=True, stop=False)
                nc.tensor.matmul(pq[:], wqk1[:, cs], xT1[:, js], start=False, stop=False)
                nc.tensor.matmul(pq[:], wqkb[:, cs], ones_row[:, js], start=False, stop=True)
                if skip128:
                    # cols 0:128 already written by emit_qk_mini2
                    nc.vector.tensor_copy(qkT[co_t][:, j * 512 + 128 : (j + 1) * 512],
                                          pq[:, 128:512])
                else:
                    nc.vector.tensor_copy(qkT[co_t][:, js], pq[:])

            def emit_qk_mini2():
                # early [*,0:128] slice of K^T so S(h0,kt0) unblocks sooner
                pq = ps_m.tile([128, 128], F32, tag="m")
                cs = slice(2 * 128, 3 * 128)
                nc.tensor.matmul(pq[:], wqk0[:, cs], xT0[:, 0:128], start=True, stop=False)
                nc.tensor.matmul(pq[:], wqk1[:, cs], xT1[:, 0:128], start=False, stop=False)
                nc.tensor.matmul(pq[:], wqkb[:, cs], ones_row[:, 0:128], start=False, stop=True)
                nc.vector.tensor_copy(qkT[2][:, 0:128], pq[:])

            def emit_v(nt):
                pv = ps_m.tile([128, 264], F32, tag="m")
                ns = slice(nt * 128, (nt + 1) * 128)
                nc.tensor.matmul(pv[:], xT0[:, ns], wv0[:], start=True, stop=False)
                nc.tensor.matmul(pv[:], xT1[:, ns], wv1[:], start=False, stop=False)
                nc.tensor.matmul(pv[:], ones_row[:, ns], wvb[:], start=False, stop=True)
                nc.vector.tensor_copy(v_sb[nt][:], pv[:])

            p_tiles = {}

            def emit_s_exp(h, kts=range(8), jsplit=False):
                g, hh = h // 4, h % 4
                qrows = slice(32 * hh, 32 * hh + 32)
                tp = (32 * hh, 0) if hh == 3 else None
                if h in p_tiles:
                    p_h = p_tiles[h]
                else:
                    p_h = [ppool.tile([128, NMAX], F32R, tag=f"p{kt}",
                                      name=f"p_h{h}_{kt}") for kt in range(8)]
                    p_tiles[h] = p_h
                for kt in kts:
                    ss = ps_s.tile([128, NMAX], F32, tag="s")
                    ks = slice(kt * 128, (kt + 1) * 128)
                    for j in range(2):
                        js = slice(j * 512, (j + 1) * 512)
                        nc.tensor.matmul(
                            ss[:, js], qkT[2 + g][qrows, ks], qkT[g][qrows, js],
                            start=True, stop=True, tile_position=tp,
                        )
                    if jsplit:
                        for j in range(2):
                            js = slice(j * 512, (j + 1) * 512)
                            nc.scalar.activation(
                                p_h[kt][:, js], ss[:, js], Exp,
                                bias=maskc[:, kt : kt + 1], scale=1.0,
                            )
                    else:
                        nc.scalar.activation(
                            p_h[kt][:], ss[:], Exp,
                            bias=maskc[:, kt : kt + 1], scale=1.0,
                        )

            def emit_av_norm(h):
                p_h = p_tiles.pop(h)
                orw = orawpool.tile([32, NMAX], F32, tag="oraw", name=f"oraw{h}")
                for j in range(2):
                    js = slice(j * 512, (j + 1) * 512)
                    po = ps_o.tile([33, 512], F32, tag="o")
                    for kt in range(8):
                        nc.tensor.matmul(
                            po[:], v_sb[kt][:, 33 * h : 33 * h + 33],
                            p_h[kt][:, js], start=(kt == 0), stop=(kt == 7),
                        )
                    rs = orawpool.tile([33, 512], F32R, tag="rs", bufs=2,
                                       name=f"rs{h}_{j}")
                    nc.vector.reciprocal(rs[32:33, :], po[32:33, :])
                    rbc = ps_m.tile([32, 512], F32, tag="m")
                    nc.tensor.matmul(rbc[:], bones[32:33, :], rs[32:33, :],
                                     start=True, stop=True)
                    nc.vector.tensor_copy(orw[:, js], po[0:32, :])
                    nc.vector.tensor_mul(oTh[h][:, js], orw[:, js], rbc[:])

            st = [stgpool.tile([128, C], F32, tag=f"st{i}", name=f"st{i}")
                  for i in range(8)]

            def emit_proj_half1(nt):
                pf = ps_m.tile([128, C], F32, tag="m")
                ns = slice(nt * 128, (nt + 1) * 128)
                for h in range(4):
                    nc.tensor.matmul(pf[:], oTh[h][:, ns], wpc[h][:],
                                     start=(h == 0), stop=(h == 3))
                nc.vector.tensor_copy(st[nt][:], pf[:])

            def emit_proj_half2(nt, fo, fcol, eng):
                pool_pick = (ps_s, "s"), (ps_m, "m"), (ps_o, "o")
                pp, ptag = pool_pick[nt % 3]
                pf = pp.tile([128, C], F32, tag=ptag)
                ns = slice(nt * 128, (nt + 1) * 128)
                for h in range(4, 8):
                    nc.tensor.matmul(pf[:], oTh[h][:, ns], wpc[h][:],
                                     start=(h == 4), stop=False)
                nc.tensor.matmul(pf[:], ones_row[:, ns], wpb[:], start=False, stop=True)
                nc.vector.tensor_add(fo[:, fcol * C : (fcol + 1) * C], pf[:], st[nt][:])
                if fcol == 1:
                    # one DMA covers two 128-row output blocks
                    dst = out_d[(nt - 1) * 128 : (nt + 1) * 128, :]
                    dst = dst.rearrange("(b p) c -> p b c", p=128)
                    src_ap = fo[:].rearrange("p (b c) -> p b c", b=2)
                    eng.dma_start(dst, src_ap)

            # ---- emission schedule (software-pipelined) ----
            # warm the ACT exp table and ramp the PE while DMAs land
            warm = cpool.tile([1, 1], F32)
            nc.vector.memset(warm[:], 0.0)
            nc.scalar.activation(warm[:], warm[:], Exp, scale=1.0)

            emit_qk(0, 0)
            emit_qk(0, 1)
            emit_qk_mini2()
            emit_s_exp(0, (0,))
            emit_qk(2, 0, skip128=True)
            emit_s_exp(0, (1,))
            emit_qk(2, 1)
            emit_v(0)
            emit_s_exp(0, (2, 3))
            emit_v(1)
            emit_v(2)
            emit_s_exp(0, (4, 5))
            emit_v(3)
            emit_v(4)
            emit_s_exp(0, (6, 7))
            emit_v(5)
            emit_v(6)
            emit_v(7)
            emit_qk(1, 0)
            emit_qk(1, 1)
            emit_qk(3, 0)
            emit_qk(3, 1)
            for h in range(H):
                if h + 1 < H:
                    emit_s_exp(h + 1)
                emit_av_norm(h)
                if h in (4, 5, 6):
                    for nt in range(3 * (h - 4), min(3 * (h - 3), 8)):
                        emit_proj_half1(nt)
            for pair in range(4):
                fo = iopool.tile([128, 2 * C], F32, tag="fo", name=f"fo{pair}")
                eng = nc.sync if pair % 2 == 0 else nc.scalar
                emit_proj_half2(2 * pair, fo, 0, eng)
                emit_proj_half2(2 * pair + 1, fo, 1, eng)

    nc.finalize()
    return nc


def _prep_shared(qkv_w, qkv_b, proj_w, proj_b):
    wq = qkv_w[:, 0:C]
    wk = qkv_w[:, C : 2 * C] * SCALE
    wv = qkv_w[:, 2 * C : 3 * C]
    bq = qkv_b[0:C]
    bk = qkv_b[C : 2 * C] * SCALE
    bv = qkv_b[2 * C : 3 * C]

    wqk = np.concatenate([wq, wk], axis=1)
    bqk = np.concatenate([bq, bk])[None, :]

    wv_aug = np.zeros((C + 1, 33 * H), dtype=np.float32)
    for h in range(H):
        wv_aug[0:C, 33 * h : 33 * h + 32] = wv[:, 32 * h : 32 * h + 32]
        wv_aug[C, 33 * h : 33 * h + 32] = bv[32 * h : 32 * h + 32]
        wv_aug[C, 33 * h + 32] = 1.0

    bones = np.zeros((33, 32), dtype=np.float32)
    bones[32, :] = 1.0

    wpa = np.concatenate([proj_w, proj_b[None, :]], axis=0)  # [257, 256]

    return {
        "wqk0": np.ascontiguousarray(wqk[0:128]),
        "wqk1": np.ascontiguousarray(wqk[128:256]),
        "wqkb": np.ascontiguousarray(bqk),
        "wv0": np.ascontiguousarray(wv_aug[0:128]),
        "wv1": np.ascontiguousarray(wv_aug[128:256]),
        "wvb": np.ascontiguousarray(wv_aug[256:257]),
        "wpa": np.ascontiguousarray(wpa),
        "ones_row": np.ones((1, NMAX), dtype=np.float32),
        "bones": bones,
    }


def _numpy_fallback(data, qkv_w, qkv_b, proj_w, proj_b, ct_mask, batch_id, pos_id):
    x = np.zeros((B, NMAX, C), dtype=np.float32)
    x[batch_id, pos_id] = data
    qkv = (x @ qkv_w + qkv_b).reshape(B, NMAX, 3, H, HD)
    q = np.moveaxis(qkv[:, :, 0], 2, 1)
    k = np.moveaxis(qkv[:, :, 1], 2, 1)
    v = np.moveaxis(qkv[:, :, 2], 2, 1)
    attn = np.einsum("bhqd,bhkd->bhqk", q * SCALE, k) + ct_mask[:, None]
    attn = attn - attn.max(axis=-1, keepdims=True)
    attn = np.exp(attn)
    attn /= attn.sum(axis=-1, keepdims=True)
    out = np.einsum("bhqk,bhkd->bhqd", attn, v)
    out = np.moveaxis(out, 1, 2).reshape(B, NMAX, C)
    out = out[batch_id, pos_id]
    return (out @ proj_w + proj_b).astype(np.float32)


def kernel(data, qkv_w, qkv_b, proj_w, proj_b, ct_mask, batch_id, pos_id,
           _profile=False):
    from concourse.bass_utils import run_bass_kernel_spmd

    data = np.asarray(data, dtype=np.float32)
    qkv_w = np.asarray(qkv_w, dtype=np.float32)
    qkv_b = np.asarray(qkv_b, dtype=np.float32)
    proj_w = np.asarray(proj_w, dtype=np.float32)
    proj_b = np.asarray(proj_b, dtype=np.float32)
    ct_mask = np.asarray(ct_mask, dtype=np.float32)
    batch_id = np.asarray(batch_id)
    pos_id = np.asarray(pos_id)

    # Device path needs the mask constant along the query axis (true for
    # padded-key masks). Otherwise fall back to a host computation.
    mask_vec = ct_mask[:, 0, :]
    if not np.array_equal(ct_mask, np.broadcast_to(mask_vec[:, None, :], ct_mask.shape)):
        return _numpy_fallback(data, qkv_w, qkv_b, proj_w, proj_b, ct_mask,
                               batch_id, pos_id)

    x = np.zeros((B, NMAX, C), dtype=np.float32)
    x[batch_id, pos_id] = data

    shared = _prep_shared(qkv_w, qkv_b, proj_w, proj_b)

    if "nc" not in _CACHE:
        _CACHE["nc"] = _build_program()
    nc = _CACHE["nc"]

    in_maps = []
    for b in range(B):
        xT = np.ascontiguousarray(x[b].T)
        im = dict(shared)
        im["xT0"] = xT[0:128]
        im["xT1"] = xT[128:256]
        im["maskc"] = np.ascontiguousarray(mask_vec[b].reshape(8, 128).T)
        in_maps.append(im)

    res = run_bass_kernel_spmd(nc, in_maps, core_ids=list(range(B)))
    if _profile:
        _CACHE["last_results"] = res

    out_pad = np.stack([res.results[b]["out"] for b in range(B)])
    return out_pad[batch_id, pos_id].astype(np.float32)


# revision 43
# speedup vs baseline: 1.0005x; 1.0005x over previous
"""Trainium2 Bass kernel for CTAttention (ragged-batch multi-head attention).

Host: pads/scatters ragged rows into [B, NMAX, C], shards batch elements
across 8 NeuronCores (batch b -> core b), compresses the additive key mask
[B, N, N] (rows are constant along the query axis) to a per-key vector.

Device (per core, one batch element):
  QKV^T = Waug^T @ x_aug        (bias folded via a ones-row, K pre-scaled)
  S^T[k, q] = K^T.T-slice @ Q^T (per head, per 128-key tile)
  P~^T = exp(S^T + mask[k])     (ScalarE, mask as per-partition bias)
  O'^T = V_aug.T @ P~^T         (V carries a ones column -> row 32 = rowsum)
  O^T = O'^T * recip(rowsum)    (selector-matmul partition broadcast)
  out = O_aug @ Wp_aug          (bias folded via the ones-row)
Matmuls run in float32r (TF32-like, full-rate fp32).
"""

import sys

sys.path.insert(0, "/opt/trn_rl_repo")

import numpy as np

B = 8
NMAX = 1024
C = 256
H = 8
HD = C // H
SCALE = HD ** -0.5

_CACHE = {}


def _build_program():
    import concourse.bass as bass
    from concourse import bacc
    import concourse.mybir as mybir
    import concourse.tile as tile

    F32 = mybir.dt.float32
    F32R = mybir.dt.float32r
    Exp = mybir.ActivationFunctionType.Exp

    nc = bacc.Bacc()

    xT0_d = nc.dram_tensor("xT0", [128, NMAX], F32, kind="ExternalInput")
    xT1_d = nc.dram_tensor("xT1", [128, NMAX], F32, kind="ExternalInput")
    ones_d = nc.dram_tensor("ones_row", [1, NMAX], F32, kind="ExternalInput")
    wqk0_d = nc.dram_tensor("wqk0", [128, 512], F32, kind="ExternalInput")
    wqk1_d = nc.dram_tensor("wqk1", [128, 512], F32, kind="ExternalInput")
    wqkb_d = nc.dram_tensor("wqkb", [1, 512], F32, kind="ExternalInput")
    wv0_d = nc.dram_tensor("wv0", [128, 264], F32, kind="ExternalInput")
    wv1_d = nc.dram_tensor("wv1", [128, 264], F32, kind="ExternalInput")
    wvb_d = nc.dram_tensor("wvb", [1, 264], F32, kind="ExternalInput")
    wpa_d = nc.dram_tensor("wpa", [C + 1, C], F32, kind="ExternalInput")
    maskc_d = nc.dram_tensor("maskc", [128, 8], F32, kind="ExternalInput")
    bones_d = nc.dram_tensor("bones", [33, 32], F32, kind="ExternalInput")
    out_d = nc.dram_tensor("out", [NMAX, C], F32, kind="ExternalOutput")

    with tile.TileContext(nc) as tc:
        with (
            nc.allow_low_precision("float32r pipeline; verified vs reference"),
            tc.tile_pool(name="const", bufs=1) as cpool,
            tc.tile_pool(name="qk", bufs=1) as qkpool,
            tc.tile_pool(name="vp", bufs=1) as vpool,
            tc.tile_pool(name="pt", bufs=2) as ppool,
            tc.tile_pool(name="orw", bufs=2) as orawpool,
            tc.tile_pool(name="ot", bufs=1) as opool,
            tc.tile_pool(name="stg", bufs=1) as stgpool,
            tc.tile_pool(name="io", bufs=4) as iopool,
            tc.tile_pool(name="ps_s", bufs=2, space="PSUM") as ps_s,
            tc.tile_pool(name="ps_o", bufs=2, space="PSUM") as ps_o,
            tc.tile_pool(name="ps_m", bufs=2, space="PSUM") as ps_m,
        ):
            # ---- constants / inputs (DMA order: QK-group-0 critical path first) ----
            xT0 = cpool.tile([128, NMAX], F32R)
            xT1 = cpool.tile([128, NMAX], F32R)
            ones_row = cpool.tile([1, NMAX], F32R)
            wqk0 = cpool.tile([128, 512], F32R)
            wqk1 = cpool.tile([128, 512], F32R)
            wqkb = cpool.tile([1, 512], F32R)
            wv0 = cpool.tile([128, 264], F32R)
            wv1 = cpool.tile([128, 264], F32R)
            wvb = cpool.tile([1, 264], F32R)
            wpc = [cpool.tile([32, C], F32R, name=f"wpc{i}") for i in range(8)]
            wpb = cpool.tile([1, C], F32R)
            maskc = cpool.tile([128, 8], F32)
            bones = cpool.tile([33, 32], F32R)
            nc.scalar.dma_start(xT0[:, 0:512], xT0_d[:, 0:512].bitcast(F32R))
            nc.sync.dma_start(wqk0[:], wqk0_d[:].bitcast(F32R))
            nc.sync.dma_start(xT0[:, 512:1024], xT0_d[:, 512:1024].bitcast(F32R))
            for eng, t, d in (
                (nc.sync, wqk1, wqk1_d), (nc.scalar, xT1, xT1_d),
                (nc.sync, wqkb, wqkb_d), (nc.scalar, ones_row, ones_d),
                (nc.gpsimd, wv0, wv0_d), (nc.gpsimd, wv1, wv1_d),
                (nc.gpsimd, wvb, wvb_d), (nc.sync, bones, bones_d),
            ):
                eng.dma_start(t[:], d[:].bitcast(F32R))
            nc.scalar.dma_start(maskc[:], maskc_d[:])
            for i in range(8):
                nc.gpsimd.dma_start(
                    wpc[i][:], wpa_d[32 * i : 32 * i + 32, :].bitcast(F32R))
            nc.gpsimd.dma_start(wpb[:], wpa_d[C : C + 1, :].bitcast(F32R))

            qkT = [qkpool.tile([128, NMAX], F32R, name=f"qkT{i}") for i in range(4)]
            v_sb = [vpool.tile([128, 264], F32R, name=f"v{i}") for i in range(8)]
            oTh = [opool.tile([32, NMAX], F32R, name=f"oTh{i}") for i in range(H)]

            def emit_qk(co_t, j, skip128=False):
                pq = ps_m.tile([128, 512], F32, tag="m")
                js = slice(j * 512, (j + 1) * 512)
                cs = slice(co_t * 128, (co_t + 1) * 128)
                nc.tensor.matmul(pq[:], wqk0[:, cs], xT0[:, js], start=True, stop=False)
                nc.tensor.matmul(pq[:], wqk1[:, cs], xT1[:, js], start=False, stop=False)
                nc.tensor.matmul(pq[:], wqkb[:, cs], ones_row[:, js], start=False, stop=True)
                if skip128:
                    # cols 0:128 already written by emit_qk_mini2
                    nc.vector.tensor_copy(qkT[co_t][:, j * 512 + 128 : (j + 1) * 512],
                                          pq[:, 128:512])
                else:
                    nc.vector.tensor_copy(qkT[co_t][:, js], pq[:])

            def emit_qk_mini2():
                # early [*,0:128] slice of K^T so S(h0,kt0) unblocks sooner
                pq = ps_m.tile([128, 128], F32, tag="m")
                cs = slice(2 * 128, 3 * 128)
                nc.tensor.matmul(pq[:], wqk0[:, cs], xT0[:, 0:128], start=True, stop=False)
                nc.tensor.matmul(pq[:], wqk1[:, cs], xT1[:, 0:128], start=False, stop=False)
                nc.tensor.matmul(pq[:], wqkb[:, cs], ones_row[:, 0:128], start=False, stop=True)
                nc.vector.tensor_copy(qkT[2][:, 0:128], pq[:])

            def emit_v(nt):
                pv = ps_m.tile([128, 264], F32, tag="m")
                ns = slice(nt * 128, (nt + 1) * 128)
                nc.tensor.matmul(pv[:], xT0[:, ns], wv0[:], start=True, stop=False)
                nc.tensor.matmul(pv[:], xT1[:, ns], wv1[:], start=False, stop=False)
                nc.tensor.matmul(pv[:], ones_row[:, ns], wvb[:], start=False, stop=True)
                nc.vector.tensor_copy(v_sb[nt][:], pv[:])

            p_tiles = {}

            def emit_s_exp(h, kts=range(8), jsplit=False):
                g, hh = h // 4, h % 4
                qrows = slice(32 * hh, 32 * hh + 32)
                tp = (32 * hh, 0) if hh == 3 else None
                if h in p_tiles:
                    p_h = p_tiles[h]
                else:
                    p_h = [ppool.tile([128, NMAX], F32R, tag=f"p{kt}",
                                      name=f"p_h{h}_{kt}") for kt in range(8)]
                    p_tiles[h] = p_h
                for kt in kts:
                    ss = ps_s.tile([128, NMAX], F32, tag="s")
                    ks = slice(kt * 128, (kt + 1) * 128)
                    for j in range(2):
                        js = slice(j * 512, (j + 1) * 512)
                        nc.tensor.matmul(
                            ss[:, js], qkT[2 + g][qrows, ks], qkT[g][qrows, js],
                            start=True, stop=True, tile_position=tp,
                        )
                    if jsplit:
                        for j in range(2):
                            js = slice(j * 512, (j + 1) * 512)
                            nc.scalar.activation(
                                p_h[kt][:, js], ss[:, js], Exp,
                                bias=maskc[:, kt : kt + 1], scale=1.0,
                            )
                    else:
                        nc.scalar.activation(
                            p_h[kt][:], ss[:], Exp,
                            bias=maskc[:, kt : kt + 1], scale=1.0,
                        )

            def emit_av_norm(h):
                p_h = p_tiles.pop(h)
                orw = orawpool.tile([32, NMAX], F32, tag="oraw", name=f"oraw{h}")
                for j in range(2):
                    js = slice(j * 512, (j + 1) * 512)
                    po = ps_o.tile([33, 512], F32, tag="o")
                    for kt in range(8):
                        nc.tensor.matmul(
                            po[:], v_sb[kt][:, 33 * h : 33 * h + 33],
                            p_h[kt][:, js], start=(kt == 0), stop=(kt == 7),
                        )
                    rs = orawpool.tile([33, 512], F32R, tag="rs", bufs=2,
                                       name=f"rs{h}_{j}")
                    nc.vector.reciprocal(rs[32:33, :], po[32:33, :])
                    rbc = ps_m.tile([32, 512], F32, tag="m")
                    nc.tensor.matmul(rbc[:], bones[32:33, :], rs[32:33, :],
                                     start=True, stop=True)
                    nc.vector.tensor_copy(orw[:, js], po[0:32, :])
                    nc.vector.tensor_mul(oTh[h][:, js], orw[:, js], rbc[:])

            st = [stgpool.tile([128, C], F32, tag=f"st{i}", name=f"st{i}")
                  for i in range(8)]

            def emit_proj_half1(nt):
                pf = ps_m.tile([128, C], F32, tag="m")
                ns = slice(nt * 128, (nt + 1) * 128)
                for h in range(4):
                    nc.tensor.matmul(pf[:], oTh[h][:, ns], wpc[h][:],
                                     start=(h == 0), stop=(h == 3))
                nc.vector.tensor_copy(st[nt][:], pf[:])

            def emit_proj_half2(nt, fo, fcol, eng):
                pool_pick = (ps_s, "s"), (ps_m, "m"), (ps_o, "o")
                pp, ptag = pool_pick[nt % 3]
                pf = pp.tile([128, C], F32, tag=ptag)
                ns = slice(nt * 128, (nt + 1) * 128)
                for h in range(4, 8):
                    nc.tensor.matmul(pf[:], oTh[h][:, ns], wpc[h][:],
                                     start=(h == 4), stop=False)
                nc.tensor.matmul(pf[:], ones_row[:, ns], wpb[:], start=False, stop=True)
                nc.vector.tensor_add(fo[:, fcol * C : (fcol + 1) * C], pf[:], st[nt][:])
                if fcol == 1:
                    # one DMA covers two 128-row output blocks
                    dst = out_d[(nt - 1) * 128 : (nt + 1) * 128, :]
                    dst = dst.rearrange("(b p) c -> p b c", p=128)
                    src_ap = fo[:].rearrange("p (b c) -> p b c", b=2)
                    eng.dma_start(dst, src_ap)

            # ---- emission schedule (software-pipelined) ----
            # warm the ACT exp table and ramp the PE while DMAs land
            warm = cpool.tile([1, 1], F32)
            nc.vector.memset(warm[:], 0.0)
            nc.scalar.activation(warm[:], warm[:], Exp, scale=1.0)

            emit_qk(0, 0)
            emit_qk(0, 1)
            emit_qk_mini2()
            emit_s_exp(0, (0,))
            emit_qk(2, 0, skip128=True)
            emit_s_exp(0, (1,))
            emit_qk(2, 1)
            emit_v(0)
            emit_s_exp(0, (2, 3))
            emit_v(1)
            emit_v(2)
            emit_s_exp(0, (4, 5))
            emit_v(3)
            emit_v(4)
            emit_s_exp(0, (6, 7))
            emit_v(5)
            emit_v(6)
            emit_v(7)
            emit_qk(1, 0)
            emit_qk(1, 1)
            emit_qk(3, 0)
            emit_qk(3, 1)
            for h in range(H):
                if h + 1 < H:
                    emit_s_exp(h + 1)
                emit_av_norm(h)
                if h in (4, 5, 6):
                    for nt in range(3 * (h - 4), min(3 * (h - 3), 8)):
                        emit_proj_half1(nt)
            for pair in range(4):
                fo = iopool.tile([128, 2 * C], F32, tag="fo", name=f"fo{pair}")
                eng = nc.sync if pair % 2 == 0 else nc.scalar
                emit_proj_half2(2 * pair, fo, 0, eng)
                emit_proj_half2(2 * pair + 1, fo, 1, eng)

    nc.finalize()
    return nc


def _prep_shared(qkv_w, qkv_b, proj_w, proj_b):
    wq = qkv_w[:, 0:C]
    wk = qkv_w[:, C : 2 * C] * SCALE
    wv = qkv_w[:, 2 * C : 3 * C]
    bq = qkv_b[0:C]
    bk = qkv_b[C : 2 * C] * SCALE
    bv = qkv_b[2 * C : 3 * C]

    wqk = np.concatenate([wq, wk], axis=1)
    bqk = np.concatenate([bq, bk])[None, :]

    wv_aug = np.zeros((C + 1, 33 * H), dtype=np.float32)
    for h in range(H):
        wv_aug[0:C, 33 * h : 33 * h + 32] = wv[:, 32 * h : 32 * h + 32]
        wv_aug[C, 33 * h : 33 * h + 32] = bv[32 * h : 32 * h + 32]
        wv_aug[C, 33 * h + 32] = 1.0

    bones = np.zeros((33, 32), dtype=np.float32)
    bones[32, :] = 1.0

    wpa = np.concatenate([proj_w, proj_b[None, :]], axis=0)  # [257, 256]

    return {
        "wqk0": np.ascontiguousarray(wqk[0:128]),
        "wqk1": np.ascontiguousarray(wqk[128:256]),
        "wqkb": np.ascontiguousarray(bqk),
        "wv0": np.ascontiguousarray(wv_aug[0:128]),
        "wv1": np.ascontiguousarray(wv_aug[128:256]),
        "wvb": np.ascontiguousarray(wv_aug[256:257]),
        "wpa": np.ascontiguousarray(wpa),
        "ones_row": np.ones((1, NMAX), dtype=np.float32),
        "bones": bones,
    }


def _numpy_fallback(data, qkv_w, qkv_b, proj_w, proj_b, ct_mask, batch_id, pos_id):
    x = np.zeros((B, NMAX, C), dtype=np.float32)
    x[batch_id, pos_id] = data
    qkv = (x @ qkv_w + qkv_b).reshape(B, NMAX, 3, H, HD)
    q = np.moveaxis(qkv[:, :, 0], 2, 1)
    k = np.moveaxis(qkv[:, :, 1], 2, 1)
    v = np.moveaxis(qkv[:, :, 2], 2, 1)
    attn = np.einsum("bhqd,bhkd->bhqk", q * SCALE, k) + ct_mask[:, None]
    attn = attn - attn.max(axis=-1, keepdims=True)
    attn = np.exp(attn)
    attn /= attn.sum(axis=-1, keepdims=True)
    out = np.einsum("bhqk,bhkd->bhqd", attn, v)
    out = np.moveaxis(out, 1, 2).reshape(B, NMAX, C)
    out = out[batch_id, pos_id]
    return (out @ proj_w + proj_b).astype(np.float32)


def kernel(data, qkv_w, qkv_b, proj_w, proj_b, ct_mask, batch_id, pos_id,
           _profile=False):
    from concourse.bass_utils import run_bass_kernel_spmd

    data = np.asarray(data, dtype=np.float32)
    qkv_w = np.asarray(qkv_w, dtype=np.float32)
    qkv_b = np.asarray(qkv_b, dtype=np.float32)
    proj_w = np.asarray(proj_w, dtype=np.float32)
    proj_b = np.asarray(proj_b, dtype=np.float32)
    ct_mask = np.asarray(ct_mask, dtype=np.float32)
    batch_id = np.asarray(batch_id)
    pos_id = np.asarray(pos_id)

    # Device path needs the mask constant along the query axis (true for
    # padded-key masks). Otherwise fall back to a host computation.
    mask_vec = ct_mask[:, 0, :]
    if not np.array_equal(ct_mask, np.broadcast_to(mask_vec[:, None, :], ct_mask.shape)):
        return _numpy_fallback(data, qkv_w, qkv_b, proj_w, proj_b, ct_mask,
                               batch_id, pos_id)

    x = np.zeros((B, NMAX, C), dtype=np.float32)
    x[batch_id, pos_id] = data

    shared = _prep_shared(qkv_w, qkv_b, proj_w, proj_b)

    if "nc" not in _CACHE:
        _CACHE["nc"] = _build_program()
    nc = _CACHE["nc"]

    in_maps = []
    for b in range(B):
        xT = np.ascontiguousarray(x[b].T)
        im = dict(shared)
        im["xT0"] = xT[0:128]
        im["xT1"] = xT[128:256]
        im["maskc"] = np.ascontiguousarray(mask_vec[b].reshape(8, 128).T)
        in_maps.append(im)

    res = run_bass_kernel_spmd(nc, in_maps, core_ids=list(range(B)))
    if _profile:
        _CACHE["last_results"] = res

    out_pad = np.stack([res.results[b]["out"] for b in range(B)])
    return out_pad[batch_id, pos_id].astype(np.float32)
